# revision 1
# baseline (speedup 1.0000x reference)
"""Trainium2 Bass kernel for cross-attention (b=4, nq=2048, nkv=1024,
qdim=1024, cdim=768, heads=16, dim_head=64).

Sharding: 8 cores = batch(4) x nq-half(2). Each core computes a disjoint
[1024, 1024] slice of the output; no collectives needed.

Per-core algorithm (all matmuls in f32r = full-rate fp32 on the PE):
  CT = ctx^T                         (PE transpose-mode, 128x128 blocks)
  KT = Wk^T @ CT   [inner, nkv]      (inner chunk p holds heads 2p, 2p+1)
  V  = (CT^T @ Wv) * mask[:, None]   [nkv, inner]  (mask folded into V)
  XT = x^T ; QT = Wq^T @ XT [inner, nq]
  per head-pair p, per nkv-chunk c:
    S^T = K_h @ Q_h^T                (row-tiled K=64: 2 heads concurrent)
    ES  = exp(SCALE * S^T)           (ScalarE, PSUM->SBUF)
    OT_acc  += V_h^T @ ES            (col-tiled M=64: 2 heads concurrent)
    rs_acc  += mask^T @ ES           (M=1 rowsum via mask column)
  rT = 1/rs ; RB = ones x rT (rank-1 broadcast) ; OT = OT_acc * RB
  out = (OT^T stacked) @ Wo + 1 x bo (bias via K=1 matmul)

Masking is multiplicative-after-exp: softmax(mask? s: -inf) == exp(s)*m /
sum(exp(s)*m), implemented by zeroing masked rows of V and using the mask
vector as the rowsum reduction weights. Scores are O(1) by construction so
unshifted exp is safe.
"""

import numpy as np
from contextlib import ExitStack

import concourse.bass as bass
import concourse.mybir as mybir
import concourse.tile as tile
from concourse import bacc
from concourse.bass_utils import run_bass_kernel_spmd
from concourse.masks import make_identity

F32 = mybir.dt.float32
F32R = mybir.dt.float32r
BF16 = mybir.dt.bfloat16
AF = mybir.ActivationFunctionType

NQ = 1024      # queries per core
NKV = 1024
QD = 1024
CD = 768
H = 16
D = 64
INNER = 1024
SCALE = D ** -0.5
P = 128
NQC = NQ // P      # 8 nq chunks
NKC = NKV // P     # 8 nkv chunks
QDC = QD // P      # 8
CDC = CD // P      # 6
HP = H // 2        # 8 head pairs


def R(ap):
    return ap.bitcast(F32R)


def _transpose_in(nc, pool, ps_pool, ident, src_d, dst, nrow_chunks, ncol_chunks):
    """Load DRAM [nrow_chunks*128, ncol_chunks*128] and write its transpose
    into dst SBUF [128, ncol_chunks * nrow] (viewed as [128, r, nrow])."""
    nrow = nrow_chunks * P
    dst3 = dst.rearrange("p (r n) -> p r n", n=nrow)
    for c in range(nrow_chunks):
        sn = pool.tile([P, ncol_chunks * P], F32R, tag="nat",
                       name=f"nat_{dst.tensor.name}_{c}")
        nc.sync.dma_start(out=sn[:], in_=src_d[c * P:(c + 1) * P, :])
        for g in range((ncol_chunks + 3) // 4):
            rlo = g * 4
            rn = min(4, ncol_chunks - rlo)
            ps = ps_pool.tile([P, 512], F32, tag="tp",
                              name=f"tp_{dst.tensor.name}_{c}_{g}")
            for j in range(rn):
                r = rlo + j
                nc.tensor.transpose(
                    R(ps[:, j * P:(j + 1) * P]),
                    sn[:, r * P:(r + 1) * P], ident[:])
            ps3 = ps.rearrange("p (r n) -> p r n", n=P)
            nc.vector.tensor_copy(
                dst3[:, rlo:rlo + rn, c * P:(c + 1) * P], ps3[:, 0:rn, :])


def _emit(tc, io):
    nc = tc.nc
    x, ctx_t, maskf, wq_d, wk_d, wv_d, wo_d, bo_d, out_d = io

    with ExitStack() as top:
        const = top.enter_context(tc.tile_pool(name="const", bufs=1))
        ident_f = const.tile([P, P], F32, tag="identf")
        make_identity(nc, ident_f)
        ident = const.tile([P, P], F32R, tag="ident")
        nc.vector.tensor_copy(ident[:], ident_f[:])
        ones_f = const.tile([P, P], F32, tag="onesf")
        nc.vector.memset(ones_f[:], 1.0)
        ones = const.tile([P, P], F32R, tag="ones")
        nc.vector.tensor_copy(ones[:], ones_f[:])
        maskp = const.tile([P, NKC], F32R, tag="maskp")  # maskp[p, c] = mask[c*128+p]
        with nc.allow_non_contiguous_dma(reason="tiny 1024-elem mask transpose"):
            nc.sync.dma_start(out=maskp[:], in_=maskf.rearrange("(c p) -> p c", p=P))
        # mask replicated to 64 columns per chunk: rowsum matmul lhsT, so the
        # per-head rowsum lands broadcast across all 64 output partitions
        mask64 = const.tile([P, NKC * D], BF16, tag="mask64")
        for c in range(NKC):
            nc.vector.tensor_copy(
                mask64[:, c * D:(c + 1) * D],
                maskp[:, c:c + 1].broadcast_to((P, D)))

        big = top.enter_context(tc.tile_pool(name="big", bufs=1))
        ot = big.tile([P, QDC * NQ], F32R, tag="ot")     # O^T: chunk k cols k*NQ..
        qt = big.tile([P, HP * NQ], F32R, tag="qt")      # Q^T: chunk p cols p*NQ..
        kt = big.tile([P, HP * NKV], F32R, tag="kt")
        vt = big.tile([P, NKC * INNER], BF16, tag="vt")  # V: chunk c cols c*INNER..

        # ------------- context side: CT -> KT, V -------------
        with tc.tile_pool(name="ct_pool", bufs=1) as ct_pool:
            ct = ct_pool.tile([P, CDC * NKV], F32R, tag="ct")
            with tc.tile_pool(name="natc", bufs=2) as natc, \
                 tc.tile_pool(name="tp_ps", bufs=2, space="PSUM") as tp_ps:
                _transpose_in(nc, natc, tp_ps, ident, ctx_t, ct, NKC, CDC)

            with tc.tile_pool(name="wk", bufs=CDC) as wk_pool, \
                 tc.tile_pool(name="pj_ps", bufs=4, space="PSUM") as pj_ps:
                wk = [wk_pool.tile([P, INNER], F32R, tag="wk", name=f"wk{k}")
                      for k in range(CDC)]
                for k in range(CDC):
                    nc.sync.dma_start(out=wk[k][:], in_=wk_d[k * P:(k + 1) * P, :])
                for p in range(HP):
                    for hf in range(2):
                        ps = pj_ps.tile([P, 512], F32, tag="pj", name=f"pjk{p}_{hf}")
                        for k in range(CDC):
                            nc.tensor.matmul(
                                ps[:], wk[k][:, p * P:(p + 1) * P],
                                ct[:, k * NKV + hf * 512: k * NKV + (hf + 1) * 512],
                                start=(k == 0), stop=(k == CDC - 1))
                        nc.vector.tensor_copy(
                            kt[:, p * NKV + hf * 512: p * NKV + (hf + 1) * 512], ps[:])

            with tc.tile_pool(name="wv", bufs=CDC) as wv_pool, \
                 tc.tile_pool(name="pj_ps2", bufs=4, space="PSUM") as pj_ps2:
                wv = [wv_pool.tile([P, INNER], F32R, tag="wv", name=f"wv{k}")
                      for k in range(CDC)]
                for k in range(CDC):
                    nc.sync.dma_start(out=wv[k][:], in_=wv_d[k * P:(k + 1) * P, :])
                for c in range(NKC):
                    for hf in range(2):
                        ps = pj_ps2.tile([P, 512], F32, tag="pj2", name=f"pjv{c}_{hf}")
                        for k in range(CDC):
                            nc.tensor.matmul(
                                ps[:], ct[:, k * NKV + c * P: k * NKV + (c + 1) * P],
                                wv[k][:, hf * 512:(hf + 1) * 512],
                                start=(k == 0), stop=(k == CDC - 1))
                        # fold key-mask into V rows (per-partition scalar)
                        nc.vector.tensor_scalar_mul(
                            vt[:, c * INNER + hf * 512: c * INNER + (hf + 1) * 512],
                            ps[:], maskp[:, c:c + 1].bitcast(F32))

        # ------------- query side: XT -> QT (two weight passes) -------------
        with tc.tile_pool(name="xt_pool", bufs=1) as xt_pool:
            xt = xt_pool.tile([P, QDC * NQ], F32R, tag="xt")
            with tc.tile_pool(name="natx", bufs=2) as natx, \
                 tc.tile_pool(name="tp_ps2", bufs=2, space="PSUM") as tp_ps2:
                _transpose_in(nc, natx, tp_ps2, ident, x, xt, NQC, QDC)

            with tc.tile_pool(name="wq", bufs=4) as wq_pool, \
                 tc.tile_pool(name="pj_ps3", bufs=4, space="PSUM") as pj_ps3:
                for half_pass in range(2):
                    wq = [wq_pool.tile([P, INNER], F32R, tag="wq",
                                       name=f"wq{half_pass}_{k}") for k in range(4)]
                    for k in range(4):
                        kk = half_pass * 4 + k
                        nc.sync.dma_start(out=wq[k][:],
                                          in_=wq_d[kk * P:(kk + 1) * P, :])
                    for p in range(HP):
                        for hf in range(2):
                            ps = pj_ps3.tile([P, 512], F32, tag="pj3",
                                             name=f"pjq{half_pass}_{p}_{hf}")
                            for k in range(4):
                                kk = half_pass * 4 + k
                                nc.tensor.matmul(
                                    ps[:], wq[k][:, p * P:(p + 1) * P],
                                    xt[:, kk * NQ + hf * 512:
                                          kk * NQ + (hf + 1) * 512],
                                    start=(k == 0), stop=(k == 3))
                            dst = qt[:, p * NQ + hf * 512: p * NQ + (hf + 1) * 512]
                            if half_pass == 0:
                                nc.vector.tensor_copy(dst, ps[:])
                            else:
                                nc.vector.tensor_add(dst, dst, ps[:])

        # ------------- attention (+ Wo/bo prefetch) -------------
        with tc.tile_pool(name="wo", bufs=QDC) as wo_pool:
            wo = [wo_pool.tile([P, QD], F32R, tag="wo", name=f"wo{k}")
                  for k in range(QDC)]
            for k in range(QDC):
                nc.sync.dma_start(out=wo[k][:], in_=wo_d[k * P:(k + 1) * P, :])

            with tc.tile_pool(name="es", bufs=3) as es_pool, \
                 tc.tile_pool(name="rt", bufs=2) as rt_pool, \
                 tc.tile_pool(name="ps_s", bufs=2, space="PSUM") as ps_s, \
                 tc.tile_pool(name="ps_o", bufs=1, space="PSUM") as ps_o, \
                 tc.tile_pool(name="ps_r", bufs=1, space="PSUM") as ps_r, \
                 tc.tile_pool(name="out_ps", bufs=2, space="PSUM") as out_ps, \
                 tc.tile_pool(name="out_sb", bufs=3) as out_sb:
                for p in range(HP):
                    for hf in range(2):
                        po = ps_o.tile([P, 512], F32, tag="po", name=f"po{p}_{hf}")
                        pr = ps_r.tile([P, 512], F32, tag="pr", name=f"pr{p}_{hf}")
                        for c in range(NKC):
                            # S^T for both heads of the pair (row-tiled K=64):
                            # head h -> cols 0:512, head h' -> cols 512:1024
                            ps = ps_s.tile([P, NQ], F32, tag="ss",
                                           name=f"ss{p}_{hf}_{c}")
                            for hh in range(2):
                                nc.tensor.matmul(
                                    ps[:, hh * 512:(hh + 1) * 512],
                                    kt[hh * D:(hh + 1) * D,
                                       p * NKV + c * P: p * NKV + (c + 1) * P],
                                    qt[hh * D:(hh + 1) * D,
                                       p * NQ + hf * 512: p * NQ + (hf + 1) * 512],
                                    start=True, stop=True,
                                    tile_position=(hh * D, 0))
                            es = es_pool.tile([P, NQ], BF16, tag="es",
                                              name=f"es{p}_{hf}_{c}")
                            nc.scalar.activation(es[:], ps[:], AF.Exp,
                                                 scale=float(SCALE))
                            for hh in range(2):
                                h = 2 * p + hh
                                esl = es[:, hh * 512:(hh + 1) * 512]
                                nc.tensor.matmul(
                                    po[hh * D:(hh + 1) * D, :],
                                    vt[:, c * INNER + h * D: c * INNER + (h + 1) * D],
                                    esl,
                                    start=(c == 0), stop=(c == NKC - 1),
                                    tile_position=(0, hh * D),
                                    skip_group_check=True)
                                nc.tensor.matmul(
                                    pr[hh * D:(hh + 1) * D, :],
                                    mask64[:, c * D:(c + 1) * D], esl,
                                    start=(c == 0), stop=(c == NKC - 1),
                                    tile_position=(0, hh * D),
                                    skip_group_check=True)
                        # epilogue: normalize this (pair, nq-half) slice
                        rt = rt_pool.tile([P, 512], F32, tag="rt",
                                          name=f"rt{p}_{hf}")
                        with nc.allow_low_precision(reason="softmax reciprocal"):
                            nc.vector.reciprocal(rt[:], pr[:])
                        nc.vector.tensor_mul(
                            ot[:, p * NQ + hf * 512: p * NQ + (hf + 1) * 512],
                            po[:], rt[:])

                # ------------- output projection (streams behind attention) ----
                bo_t = out_sb.tile([1, QD], F32R, tag="bo", bufs=1)
                nc.sync.dma_start(out=bo_t[:],
                                  in_=bo_d[:].rearrange("(o n) -> o n", o=1))
                for m in range(NQC):
                    for n in range(2):
                        ps = out_ps.tile([P, 512], F32, tag="ops", name=f"ops{m}_{n}")
                        for k in range(QDC):
                            nc.tensor.matmul(
                                ps[:],
                                ot[:, k * NQ + m * P: k * NQ + (m + 1) * P],
                                wo[k][:, n * 512:(n + 1) * 512],
                                start=(k == 0), stop=False,
                                skip_group_check=True)
                        nc.tensor.matmul(
                            ps[:], ones[0:1, 0:P],
                            bo_t[0:1, n * 512:(n + 1) * 512],
                            start=False, stop=True, skip_group_check=True)
                        sb = out_sb.tile([P, 512], F32, tag="osb", name=f"osb{m}_{n}")
                        nc.vector.tensor_copy(sb[:], ps[:])
                        nc.sync.dma_start(
                            out=out_d[m * P:(m + 1) * P, n * 512:(n + 1) * 512],
                            in_=sb[:])


_CACHED = {}


def _build(iters=1):
    if iters in _CACHED:
        return _CACHED[iters]
    nc = bacc.Bacc("TRN2", debug=False, target_bir_lowering=False)
    x = nc.dram_tensor("x", [NQ, QD], F32R, kind="ExternalInput").ap()
    ctx_t = nc.dram_tensor("ctx", [NKV, CD], F32R, kind="ExternalInput").ap()
    maskf = nc.dram_tensor("maskf", [NKV], F32R, kind="ExternalInput").ap()
    wq_d = nc.dram_tensor("wq", [QD, INNER], F32R, kind="ExternalInput").ap()
    wk_d = nc.dram_tensor("wk", [CD, INNER], F32R, kind="ExternalInput").ap()
    wv_d = nc.dram_tensor("wv", [CD, INNER], F32R, kind="ExternalInput").ap()
    wo_d = nc.dram_tensor("wo", [INNER, QD], F32R, kind="ExternalInput").ap()
    bo_d = nc.dram_tensor("bo", [QD], F32R, kind="ExternalInput").ap()
    out_d = nc.dram_tensor("out", [NQ, QD], F32, kind="ExternalOutput").ap()
    io = (x, ctx_t, maskf, wq_d, wk_d, wv_d, wo_d, bo_d, out_d)
    with tile.TileContext(nc) as tc:
        for _ in range(iters):
            _emit(tc, io)
    nc.compile()
    _CACHED[iters] = nc
    return nc


def make_in_maps(x, context, mask, Wq, Wk, Wv, Wo, bo):
    x = np.asarray(x, dtype=np.float32)
    context = np.asarray(context, dtype=np.float32)
    maskf = np.asarray(mask).astype(np.float32)
    Wq = np.ascontiguousarray(np.asarray(Wq, dtype=np.float32))
    Wk = np.ascontiguousarray(np.asarray(Wk, dtype=np.float32))
    Wv = np.ascontiguousarray(np.asarray(Wv, dtype=np.float32))
    Wo = np.ascontiguousarray(np.asarray(Wo, dtype=np.float32))
    bo = np.ascontiguousarray(np.asarray(bo, dtype=np.float32))
    in_maps = []
    for b in range(4):
        for qh in range(2):
            in_maps.append({
                "x": np.ascontiguousarray(x[b, qh * NQ:(qh + 1) * NQ, :]),
                "ctx": np.ascontiguousarray(context[b]),
                "maskf": np.ascontiguousarray(maskf[b]),
                "wq": Wq, "wk": Wk, "wv": Wv, "wo": Wo, "bo": bo,
            })
    return in_maps


def run_sharded(x, context, mask, Wq, Wk, Wv, Wo, bo, trace=False, **kw):
    nc = _build()
    in_maps = make_in_maps(x, context, mask, Wq, Wk, Wv, Wo, bo)
    res = run_bass_kernel_spmd(nc, in_maps, list(range(8)), trace=trace, **kw)
    out = np.empty((4, 2 * NQ, QD), dtype=np.float32)
    for i in range(8):
        b, qh = divmod(i, 2)
        out[b, qh * NQ:(qh + 1) * NQ, :] = res.results[i]["out"]
    return out, res


def kernel(x, context, mask, Wq, Wk, Wv, Wo, bo):
    out, _ = run_sharded(x, context, mask, Wq, Wk, Wv, Wo, bo, trace=False)
    return out

